# revision 2
# baseline (speedup 1.0000x reference)
"""Trainium2 Bass kernel for nn_CorrClassLoss.

Reference computation (B=4, C=19, H=512, W=1024, N=5000, IGNORE=255):
  ref_class = argmax_c inputs_ref[b].reshape(C, H*W)      # flat W-major
  lin_ref   = 512*y_ref + x_ref    (NOTE: linearized with H, kept faithfully)
  lin_other = 512*y_other + x_other
  gathered  = ref_class[b, lin_ref]
  target[b, lin_other] = gathered  (scatter, last write wins; rest IGNORE)
  loss = mean over non-ignored pixels of -log_softmax(inputs_other)[b, target, px]

Since lin = 512*y + x with x,y in [0,512), only flat positions [0, 262144)
are ever touched, and at most N unique scatter destinations per batch
contribute to the loss:

  loss = -(1/cnt) * sum over unique dests d (last writer j, src s_j) of
         [ x_other[b, cls(s_j), d] - ln(sum_c exp(x_other[b, c, d])) ]
  cls(s) = argmax_c x_ref[b, c, s],  cnt = total unique dests.

Strategy (8 cores, data-parallel over (batch, half-of-correspondences)):
  Host does index-only math (dedup last-wins, split j by the pixel-half of
  s_j, pack padded gather-offset tables) and hands each core a single
  pixel-major fp16 tensor cat_t = [ref_half_t; other_t; zero-row] (a
  layout/sharding choice; all value compute happens on device).
  Device per core: ONE indirect gather (multi-column offset table read
  straight from DRAM) fetches the ref vector at s_j and the other vector
  at d_j for every correspondence; pad slots point at the zero row so no
  masking/memset is needed.  Argmax one-hot via grouped max + is_ge;
  t1 = onehot . other_vec;  t2 = ln(sum_c exp(other_vec[c])).
  Output [P, 1] = per-partition sums of (t1 - t2); host sums partitions,
  adds back the pads' exactly-known -ln(19) contribution, and divides.
"""

import sys

if "/opt/trn_rl_repo" not in sys.path:
    sys.path.insert(0, "/opt/trn_rl_repo")

import numpy as np

B, C, H, W = 4, 19, 512, 1024
HW = H * W                 # 524288
NPIX = 262144              # touched flat range [0, 262144)
NPIX_H = NPIX // 2         # 131072 source pixels per core
N = 5000
NCORES = 8

P = 128                    # partitions
M = NPIX_H + NPIX + 1      # cat_t rows: ref half + other + one zero row
ZERO_OFF = (NPIX_H + NPIX) * 19   # element offset of the zero row

_programs = {}


def _build_program(cgg):
    import concourse.bass as bass
    import concourse.bacc as bacc
    import concourse.mybir as mybir
    import concourse.tile as tile

    GW = cgg * 19

    nc = bacc.Bacc("TRN2", target_bir_lowering=False, debug=False,
                   num_devices=NCORES)

    # fp16 pixel-major shards: [ref half (NPIX_H); other (NPIX); zeros (1)]
    cat_t = nc.dram_tensor("cat_t", [M, C], mybir.dt.float16,
                           kind="ExternalInput")
    # gather offsets: cols [0,cgg) = s_local*19, cols [cgg,2cgg) =
    # (NPIX_H+d)*19; element j at [j%P, j//P]; pads -> ZERO_OFF
    offs = nc.dram_tensor("offs", [P, 2 * cgg], mybir.dt.int32,
                          kind="ExternalInput")
    out = nc.dram_tensor("out", [P, 1], mybir.dt.float32,
                         kind="ExternalOutput")

    cat_flat = cat_t.rearrange("p c -> (p c)")[:, None]

    with tile.TileContext(nc) as tc:
        with tc.tile_pool(name="gb", bufs=1) as gb:
            # one gather for everything: ref vectors land in G[:, :GW],
            # other vectors in G[:, GW:]; pad slots read the zero row
            G = gb.tile([P, 2 * GW], mybir.dt.float16)
            nc.gpsimd.indirect_dma_start(
                out=G[:],
                out_offset=None,
                in_=cat_flat,
                in_offset=bass.IndirectOffsetOnAxis(ap=offs[:, :], axis=0),
                bounds_check=M * 19 - 1,
                oob_is_err=False,
            )

            Rv = G[:, 0:GW].rearrange("p (g c) -> p g c", c=19)
            R2 = G[:, GW:2 * GW]
            m2 = gb.tile([P, cgg], mybir.dt.float16)
            nc.vector.tensor_reduce(out=m2[:], in_=Rv,
                                    axis=mybir.AxisListType.X,
                                    op=mybir.AluOpType.max)
            eq = gb.tile([P, GW], mybir.dt.float16)
            eqv = eq[:].rearrange("p (g c) -> p g c", c=19)
            nc.vector.tensor_tensor(
                out=eqv, in0=Rv,
                in1=m2[:, :, None].to_broadcast([P, cgg, 19]),
                op=mybir.AluOpType.is_ge,
            )
            nc.vector.tensor_tensor(out=eq[:], in0=eq[:], in1=R2,
                                    op=mybir.AluOpType.mult)
            t1 = gb.tile([P, cgg], mybir.dt.float32)
            nc.vector.tensor_reduce(out=t1[:], in_=eqv,
                                    axis=mybir.AxisListType.X,
                                    op=mybir.AluOpType.add)

            e2 = gb.tile([P, GW], mybir.dt.float32)
            nc.scalar.activation(e2[:], R2,
                                 mybir.ActivationFunctionType.Exp)
            S2 = gb.tile([P, cgg], mybir.dt.float32)
            nc.vector.tensor_reduce(
                out=S2[:],
                in_=e2[:].rearrange("p (g c) -> p g c", c=19),
                axis=mybir.AxisListType.X, op=mybir.AluOpType.add)
            L2 = gb.tile([P, cgg], mybir.dt.float32)
            nc.scalar.activation(L2[:], S2[:],
                                 mybir.ActivationFunctionType.Ln)

            nc.vector.tensor_tensor(out=t1[:], in0=t1[:], in1=L2[:],
                                    op=mybir.AluOpType.subtract)
            vr = gb.tile([P, 1], mybir.dt.float32)
            nc.vector.tensor_reduce(out=vr[:], in_=t1[:],
                                    axis=mybir.AxisListType.X,
                                    op=mybir.AluOpType.add)
            nc.sync.dma_start(out=out[:, :], in_=vr[:])

    nc.finalize()
    return nc


def _get_program(cgg):
    if cgg not in _programs:
        _programs[cgg] = _build_program(cgg)
    return _programs[cgg]


def _host_prep(inds_ref, inds_other):
    """Index-only host math: dedup scatter (last wins), partition per core."""
    ir = np.asarray(inds_ref).astype(np.int64)      # [B, 2, N]
    io = np.asarray(inds_other).astype(np.int64)
    valid = ((ir[:, 0] >= 0) & (ir[:, 0] < W) & (ir[:, 1] >= 0) & (ir[:, 1] < H)
             & (io[:, 0] >= 0) & (io[:, 0] < W) & (io[:, 1] >= 0)
             & (io[:, 1] < H))                       # [B, N]
    lin_ref = H * ir[:, 1] + ir[:, 0]                # [B, N]
    lin_other = H * io[:, 1] + io[:, 0]

    per_core = []
    count = 0
    for b in range(B):
        v = valid[b]
        lo = lin_other[b][v]
        lr = np.clip(lin_ref[b][v], 0, HW - 1)
        # last-write-wins dedup on destinations
        u, first_rev = np.unique(lo[::-1], return_index=True)
        last_idx = len(lo) - 1 - first_rev
        d_arr = u.astype(np.int64)
        s_arr = lr[last_idx].astype(np.int64)
        count += len(u)
        for h in range(2):
            sel = (s_arr // NPIX_H) == h
            s_local = s_arr[sel] - h * NPIX_H
            d_sel = d_arr[sel]
            per_core.append({
                "b": b, "h": h,
                "s": s_local, "d": d_sel,
            })
    return per_core, count


def _pack_offs(pc, cgg):
    offs = np.full((P, 2 * cgg), ZERO_OFF, dtype=np.int32)
    s, d = pc["s"], pc["d"]
    n = len(s)
    assert n <= cgg * P
    jj = np.arange(n)
    offs[jj % P, jj // P] = s * 19
    offs[jj % P, cgg + jj // P] = (NPIX_H + d) * 19
    return offs


def _make_in_maps(inputs_ref, inputs_other, per_core, cgg):
    ref_flat = inputs_ref.reshape(B, C, HW)
    other_flat = inputs_other.reshape(B, C, HW)
    other_cache = {}
    zrow = np.zeros((1, C), dtype=np.float16)
    in_maps = []
    for pc in per_core:
        b, h = pc["b"], pc["h"]
        ref_td = np.ascontiguousarray(
            ref_flat[b, :, h * NPIX_H:(h + 1) * NPIX_H].T).astype(np.float16)
        if b not in other_cache:
            other_cache[b] = np.ascontiguousarray(
                other_flat[b, :, :NPIX].T).astype(np.float16)
        cat = np.concatenate([ref_td, other_cache[b], zrow], axis=0)
        in_maps.append({
            "cat_t": cat,
            "offs": _pack_offs(pc, cgg),
        })
    return in_maps


def kernel(inputs_ref, inputs_other, inds_ref, inds_other, weights):
    from concourse.bass_utils import run_bass_kernel_spmd

    inputs_ref = np.asarray(inputs_ref, dtype=np.float32)
    inputs_other = np.asarray(inputs_other, dtype=np.float32)

    per_core, count = _host_prep(inds_ref, inds_other)
    # exact-fit capacity: compile (and cache) the program for the actual
    # worst-core correspondence count, rounded up to whole 128-columns
    max_n = max(len(pc["s"]) for pc in per_core)
    cgg = max(1, -(-max_n // P))
    nc = _get_program(cgg)

    in_maps = _make_in_maps(inputs_ref, inputs_other, per_core, cgg)
    res = run_bass_kernel_spmd(nc, in_maps, core_ids=list(range(NCORES)))
    total = 0.0
    ln19 = float(np.log(np.float32(19.0)))
    for pc, r in zip(per_core, res.results):
        o = np.asarray(r["out"], dtype=np.float64)
        n_pad = cgg * P - len(pc["s"])
        total += o.sum() + n_pad * ln19
    loss = -total / max(count, 1)
    return np.float32(loss)


# revision 5
# speedup vs baseline: 6.3952x; 6.3952x over previous
"""Trainium2 Bass kernel for nn_CorrClassLoss.

Reference computation (B=4, C=19, H=512, W=1024, N=5000, IGNORE=255):
  ref_class = argmax_c inputs_ref[b].reshape(C, H*W)      # flat W-major
  lin_ref   = 512*y_ref + x_ref    (NOTE: linearized with H, kept faithfully)
  lin_other = 512*y_other + x_other
  gathered  = ref_class[b, lin_ref]
  target[b, lin_other] = gathered  (scatter, last write wins; rest IGNORE)
  loss = mean over non-ignored pixels of -log_softmax(inputs_other)[b, target, px]

Since lin = 512*y + x with x,y in [0,512), only flat positions [0, 262144)
are ever touched, and at most N unique scatter destinations per batch
contribute to the loss:

  loss = -(1/cnt) * sum over unique dests d (last writer j, src s_j) of
         [ x_other[b, cls(s_j), d] - ln(sum_c exp(x_other[b, c, d])) ]
  cls(s) = argmax_c x_ref[b, c, s],  cnt = total unique dests.

Strategy (8 cores, data-parallel over (batch, half-of-correspondences)):
  Host does index-only math (dedup last-wins, split j by the pixel-half of
  s_j, pack padded gather-offset tables) and hands each core a single
  pixel-major fp16 tensor cat_t = [ref_half_t; other_t; zero-row] (a
  layout/sharding choice; all value compute happens on device).
  Device per core: ONE indirect gather (multi-column offset table read
  straight from DRAM) fetches the ref vector at s_j and the other vector
  at d_j for every correspondence; pad slots point at the zero row so no
  masking/memset is needed.  Argmax one-hot via grouped max + is_ge;
  t1 = onehot . other_vec;  t2 = ln(sum_c exp(other_vec[c])).
  Output [P, 1] = per-partition sums of (t1 - t2); host sums partitions,
  adds back the pads' exactly-known -ln(19) contribution, and divides.
"""

import sys

if "/opt/trn_rl_repo" not in sys.path:
    sys.path.insert(0, "/opt/trn_rl_repo")

import numpy as np

B, C, H, W = 4, 19, 512, 1024
HW = H * W                 # 524288
NPIX = 262144              # touched flat range [0, 262144)
NPIX_H = NPIX // 2         # 131072 source pixels per core
N = 5000
NCORES = 8

P = 128                    # partitions
M = NPIX_H + NPIX + 1      # cat_t rows: ref half + other + one zero row
ZERO_ROW = NPIX_H + NPIX          # row index of the zero row

_programs = {}


def _build_program(cgg):
    import concourse.bass as bass
    import concourse.bacc as bacc
    import concourse.mybir as mybir
    import concourse.tile as tile

    GW = cgg * 19

    nc = bacc.Bacc("TRN2", target_bir_lowering=False, debug=False,
                   num_devices=NCORES)

    # fp16 pixel-major shards: [ref half (NPIX_H); other (NPIX); zeros (1)]
    cat_t = nc.dram_tensor("cat_t", [M, C], mybir.dt.float16,
                           kind="ExternalInput")
    # gather offsets (row indices into cat_t): cols [0,cgg) = s_local,
    # cols [cgg,2cgg) = NPIX_H+d; element j at [j%P, j//P]; pads -> ZERO_ROW
    offs = nc.dram_tensor("offs", [P, 2 * cgg], mybir.dt.int32,
                          kind="ExternalInput")
    out = nc.dram_tensor("out", [P, 1], mybir.dt.float32,
                         kind="ExternalOutput")

    with tile.TileContext(nc) as tc:
        with tc.tile_pool(name="gb", bufs=1) as gb:
            # one gather for everything: ref vectors land in G[:, :GW],
            # other vectors in G[:, GW:]; pad slots read the zero row.
            # in_ is the raw 2D tensor (contiguous dims merge in the AP) so
            # each partition's 2*GW-element row is one modeled descriptor.
            G = gb.tile([P, 2 * GW], mybir.dt.float16)
            nc.gpsimd.indirect_dma_start(
                out=G[:],
                out_offset=None,
                in_=cat_t[:, :],
                in_offset=bass.IndirectOffsetOnAxis(ap=offs[:, :], axis=0),
                bounds_check=M - 1,
                oob_is_err=False,
            )

            Rv = G[:, 0:GW].rearrange("p (g c) -> p g c", c=19)
            R2 = G[:, GW:2 * GW]
            m2 = gb.tile([P, cgg], mybir.dt.float16)
            nc.vector.tensor_reduce(out=m2[:], in_=Rv,
                                    axis=mybir.AxisListType.X,
                                    op=mybir.AluOpType.max)
            eq = gb.tile([P, GW], mybir.dt.float16)
            eqv = eq[:].rearrange("p (g c) -> p g c", c=19)
            nc.vector.tensor_tensor(
                out=eqv, in0=Rv,
                in1=m2[:, :, None].to_broadcast([P, cgg, 19]),
                op=mybir.AluOpType.is_ge,
            )
            nc.vector.tensor_tensor(out=eq[:], in0=eq[:], in1=R2,
                                    op=mybir.AluOpType.mult)
            t1 = gb.tile([P, cgg], mybir.dt.float32)
            nc.vector.tensor_reduce(out=t1[:], in_=eqv,
                                    axis=mybir.AxisListType.X,
                                    op=mybir.AluOpType.add)

            e2 = gb.tile([P, GW], mybir.dt.float32)
            nc.scalar.activation(e2[:], R2,
                                 mybir.ActivationFunctionType.Exp)
            S2 = gb.tile([P, cgg], mybir.dt.float32)
            nc.vector.tensor_reduce(
                out=S2[:],
                in_=e2[:].rearrange("p (g c) -> p g c", c=19),
                axis=mybir.AxisListType.X, op=mybir.AluOpType.add)
            L2 = gb.tile([P, cgg], mybir.dt.float32)
            nc.scalar.activation(L2[:], S2[:],
                                 mybir.ActivationFunctionType.Ln)

            nc.vector.tensor_tensor(out=t1[:], in0=t1[:], in1=L2[:],
                                    op=mybir.AluOpType.subtract)
            vr = gb.tile([P, 1], mybir.dt.float32)
            nc.vector.tensor_reduce(out=vr[:], in_=t1[:],
                                    axis=mybir.AxisListType.X,
                                    op=mybir.AluOpType.add)
            nc.sync.dma_start(out=out[:, :], in_=vr[:])

    nc.finalize()
    return nc


def _get_program(cgg):
    if cgg not in _programs:
        _programs[cgg] = _build_program(cgg)
    return _programs[cgg]


def _host_prep(inds_ref, inds_other):
    """Index-only host math: dedup scatter (last wins), partition per core."""
    ir = np.asarray(inds_ref).astype(np.int64)      # [B, 2, N]
    io = np.asarray(inds_other).astype(np.int64)
    valid = ((ir[:, 0] >= 0) & (ir[:, 0] < W) & (ir[:, 1] >= 0) & (ir[:, 1] < H)
             & (io[:, 0] >= 0) & (io[:, 0] < W) & (io[:, 1] >= 0)
             & (io[:, 1] < H))                       # [B, N]
    lin_ref = H * ir[:, 1] + ir[:, 0]                # [B, N]
    lin_other = H * io[:, 1] + io[:, 0]

    per_core = []
    count = 0
    for b in range(B):
        v = valid[b]
        lo = lin_other[b][v]
        lr = np.clip(lin_ref[b][v], 0, HW - 1)
        # last-write-wins dedup on destinations
        u, first_rev = np.unique(lo[::-1], return_index=True)
        last_idx = len(lo) - 1 - first_rev
        d_arr = u.astype(np.int64)
        s_arr = lr[last_idx].astype(np.int64)
        count += len(u)
        for h in range(2):
            sel = (s_arr // NPIX_H) == h
            s_local = s_arr[sel] - h * NPIX_H
            d_sel = d_arr[sel]
            per_core.append({
                "b": b, "h": h,
                "s": s_local, "d": d_sel,
            })
    return per_core, count


def _pack_offs(pc, cgg):
    offs = np.full((P, 2 * cgg), ZERO_ROW, dtype=np.int32)
    s, d = pc["s"], pc["d"]
    n = len(s)
    assert n <= cgg * P
    jj = np.arange(n)
    offs[jj % P, jj // P] = s
    offs[jj % P, cgg + jj // P] = NPIX_H + d
    return offs


def _make_in_maps(inputs_ref, inputs_other, per_core, cgg):
    ref_flat = inputs_ref.reshape(B, C, HW)
    other_flat = inputs_other.reshape(B, C, HW)
    other_cache = {}
    zrow = np.zeros((1, C), dtype=np.float16)
    in_maps = []
    for pc in per_core:
        b, h = pc["b"], pc["h"]
        ref_td = np.ascontiguousarray(
            ref_flat[b, :, h * NPIX_H:(h + 1) * NPIX_H].T).astype(np.float16)
        if b not in other_cache:
            other_cache[b] = np.ascontiguousarray(
                other_flat[b, :, :NPIX].T).astype(np.float16)
        cat = np.concatenate([ref_td, other_cache[b], zrow], axis=0)
        in_maps.append({
            "cat_t": cat,
            "offs": _pack_offs(pc, cgg),
        })
    return in_maps


def kernel(inputs_ref, inputs_other, inds_ref, inds_other, weights):
    from concourse.bass_utils import run_bass_kernel_spmd

    inputs_ref = np.asarray(inputs_ref, dtype=np.float32)
    inputs_other = np.asarray(inputs_other, dtype=np.float32)

    per_core, count = _host_prep(inds_ref, inds_other)
    # exact-fit capacity: compile (and cache) the program for the actual
    # worst-core correspondence count, rounded up to whole 128-columns
    max_n = max(len(pc["s"]) for pc in per_core)
    cgg = max(1, -(-max_n // P))
    nc = _get_program(cgg)

    in_maps = _make_in_maps(inputs_ref, inputs_other, per_core, cgg)
    res = run_bass_kernel_spmd(nc, in_maps, core_ids=list(range(NCORES)))
    total = 0.0
    ln19 = float(np.log(np.float32(19.0)))
    for pc, r in zip(per_core, res.results):
        o = np.asarray(r["out"], dtype=np.float64)
        n_pad = cgg * P - len(pc["s"])
        total += o.sum() + n_pad * ln19
    loss = -total / max(count, 1)
    return np.float32(loss)


# revision 8
# speedup vs baseline: 8.5932x; 1.3437x over previous
"""Trainium2 Bass kernel for nn_CorrClassLoss.

Reference computation (B=4, C=19, H=512, W=1024, N=5000, IGNORE=255):
  ref_class = argmax_c inputs_ref[b].reshape(C, H*W)      # flat W-major
  lin_ref   = 512*y_ref + x_ref    (NOTE: linearized with H, kept faithfully)
  lin_other = 512*y_other + x_other
  gathered  = ref_class[b, lin_ref]
  target[b, lin_other] = gathered  (scatter, last write wins; rest IGNORE)
  loss = mean over non-ignored pixels of -log_softmax(inputs_other)[b, target, px]

Since lin = 512*y + x with x,y in [0,512), only flat positions [0, 262144)
are ever touched, and at most N unique scatter destinations per batch
contribute to the loss:

  loss = -(1/cnt) * sum over unique dests d (last writer j, src s_j) of
         [ x_other[b, cls(s_j), d] - ln(sum_c exp(x_other[b, c, d])) ]
  cls(s) = argmax_c x_ref[b, c, s],  cnt = total unique dests.

Strategy (8 cores, data-parallel over (batch, half-of-correspondences)):
  Host does index-only math (dedup last-wins, split j by the pixel-half of
  s_j, pack padded gather-offset tables) and hands each core a single
  pixel-major fp16 tensor cat_t = [ref_half_t; other_t; zero-row] (a
  layout/sharding choice; all value compute happens on device).
  Device per core: ONE indirect gather (multi-column offset table read
  straight from DRAM) fetches the ref vector at s_j and the other vector
  at d_j for every correspondence; pad slots point at the zero row so no
  masking/memset is needed.  Argmax one-hot via grouped max + is_ge;
  t1 = onehot . other_vec;  t2 = ln(sum_c exp(other_vec[c])).
  Output [P, 1] = per-partition sums of (t1 - t2); host sums partitions,
  adds back the pads' exactly-known -ln(19) contribution, and divides.
"""

import sys

if "/opt/trn_rl_repo" not in sys.path:
    sys.path.insert(0, "/opt/trn_rl_repo")

import numpy as np

B, C, H, W = 4, 19, 512, 1024
HW = H * W                 # 524288
NPIX = 262144              # touched flat range [0, 262144)
NPIX_H = NPIX // 2         # 131072 source pixels per core
N = 5000
NCORES = 8

P = 128                    # partitions
M = NPIX_H + NPIX + 1      # cat_t rows: ref half + other + one zero row
ZERO_ROW = NPIX_H + NPIX          # row index of the zero row

_programs = {}


def _build_program(cgg):
    import concourse.bass as bass
    import concourse.bacc as bacc
    import concourse.mybir as mybir
    import concourse.tile as tile

    GW = cgg * 19

    nc = bacc.Bacc("TRN2", target_bir_lowering=False, debug=False,
                   num_devices=NCORES)

    # fp16 pixel-major shards: [ref half (NPIX_H); other (NPIX); zeros (1)]
    cat_t = nc.dram_tensor("cat_t", [M, C], mybir.dt.float16,
                           kind="ExternalInput")
    # gather offsets (row indices into cat_t): cols [0,cgg) = s_local,
    # cols [cgg,2cgg) = NPIX_H+d; element j at [j%P, j//P]; pads -> ZERO_ROW
    offs = nc.dram_tensor("offs", [P, 2 * cgg], mybir.dt.int32,
                          kind="ExternalInput")
    out = nc.dram_tensor("out", [P, 1], mybir.dt.float32,
                         kind="ExternalOutput")

    cat_flat = cat_t.rearrange("p c -> (p c)")

    with tile.TileContext(nc) as tc:
        with tc.tile_pool(name="gb", bufs=1) as gb:
            # one gather for everything: ref vectors land in G[:, :GW],
            # other vectors in G[:, GW:]; pad slots read the zero row.
            # in_ is the flat 1D view (one contiguous run) so each
            # partition's 2*GW-element row is one modeled descriptor.
            G = gb.tile([P, 2 * GW], mybir.dt.float16)
            nc.gpsimd.indirect_dma_start(
                out=G[:],
                out_offset=None,
                in_=cat_flat[None, :],
                in_offset=bass.IndirectOffsetOnAxis(ap=offs[:, :], axis=1),
                bounds_check=M * 19 - 1,
                oob_is_err=False,
            )

            Rv = G[:, 0:GW].rearrange("p (g c) -> p g c", c=19)
            R2 = G[:, GW:2 * GW]
            m2 = gb.tile([P, cgg], mybir.dt.float16)
            nc.vector.tensor_reduce(out=m2[:], in_=Rv,
                                    axis=mybir.AxisListType.X,
                                    op=mybir.AluOpType.max)
            eq = gb.tile([P, GW], mybir.dt.float16)
            eqv = eq[:].rearrange("p (g c) -> p g c", c=19)
            nc.vector.tensor_tensor(
                out=eqv, in0=Rv,
                in1=m2[:, :, None].to_broadcast([P, cgg, 19]),
                op=mybir.AluOpType.is_ge,
            )
            nc.vector.tensor_tensor(out=eq[:], in0=eq[:], in1=R2,
                                    op=mybir.AluOpType.mult)
            t1 = gb.tile([P, cgg], mybir.dt.float32)
            nc.vector.tensor_reduce(out=t1[:], in_=eqv,
                                    axis=mybir.AxisListType.X,
                                    op=mybir.AluOpType.add)

            e2 = gb.tile([P, GW], mybir.dt.float32)
            nc.scalar.activation(e2[:], R2,
                                 mybir.ActivationFunctionType.Exp)
            S2 = gb.tile([P, cgg], mybir.dt.float32)
            nc.vector.tensor_reduce(
                out=S2[:],
                in_=e2[:].rearrange("p (g c) -> p g c", c=19),
                axis=mybir.AxisListType.X, op=mybir.AluOpType.add)
            L2 = gb.tile([P, cgg], mybir.dt.float32)
            nc.scalar.activation(L2[:], S2[:],
                                 mybir.ActivationFunctionType.Ln)

            nc.vector.tensor_tensor(out=t1[:], in0=t1[:], in1=L2[:],
                                    op=mybir.AluOpType.subtract)
            vr = gb.tile([P, 1], mybir.dt.float32)
            nc.vector.tensor_reduce(out=vr[:], in_=t1[:],
                                    axis=mybir.AxisListType.X,
                                    op=mybir.AluOpType.add)
            nc.sync.dma_start(out=out[:, :], in_=vr[:])

    nc.finalize()
    return nc


def _get_program(cgg):
    if cgg not in _programs:
        _programs[cgg] = _build_program(cgg)
    return _programs[cgg]


def _host_prep(inds_ref, inds_other):
    """Index-only host math: dedup scatter (last wins), partition per core."""
    ir = np.asarray(inds_ref).astype(np.int64)      # [B, 2, N]
    io = np.asarray(inds_other).astype(np.int64)
    valid = ((ir[:, 0] >= 0) & (ir[:, 0] < W) & (ir[:, 1] >= 0) & (ir[:, 1] < H)
             & (io[:, 0] >= 0) & (io[:, 0] < W) & (io[:, 1] >= 0)
             & (io[:, 1] < H))                       # [B, N]
    lin_ref = H * ir[:, 1] + ir[:, 0]                # [B, N]
    lin_other = H * io[:, 1] + io[:, 0]

    per_core = []
    count = 0
    for b in range(B):
        v = valid[b]
        lo = lin_other[b][v]
        lr = np.clip(lin_ref[b][v], 0, HW - 1)
        # last-write-wins dedup on destinations
        u, first_rev = np.unique(lo[::-1], return_index=True)
        last_idx = len(lo) - 1 - first_rev
        d_arr = u.astype(np.int64)
        s_arr = lr[last_idx].astype(np.int64)
        count += len(u)
        for h in range(2):
            sel = (s_arr // NPIX_H) == h
            s_local = s_arr[sel] - h * NPIX_H
            d_sel = d_arr[sel]
            per_core.append({
                "b": b, "h": h,
                "s": s_local, "d": d_sel,
            })
    return per_core, count


def _pack_offs(pc, cgg):
    offs = np.full((P, 2 * cgg), ZERO_ROW * 19, dtype=np.int32)
    s, d = pc["s"], pc["d"]
    n = len(s)
    assert n <= cgg * P
    jj = np.arange(n)
    offs[jj % P, jj // P] = s * 19
    offs[jj % P, cgg + jj // P] = (NPIX_H + d) * 19
    return offs


def _make_in_maps(inputs_ref, inputs_other, per_core, cgg):
    ref_flat = inputs_ref.reshape(B, C, HW)
    other_flat = inputs_other.reshape(B, C, HW)
    other_cache = {}
    zrow = np.zeros((1, C), dtype=np.float16)
    in_maps = []
    for pc in per_core:
        b, h = pc["b"], pc["h"]
        ref_td = np.ascontiguousarray(
            ref_flat[b, :, h * NPIX_H:(h + 1) * NPIX_H].T).astype(np.float16)
        if b not in other_cache:
            other_cache[b] = np.ascontiguousarray(
                other_flat[b, :, :NPIX].T).astype(np.float16)
        cat = np.concatenate([ref_td, other_cache[b], zrow], axis=0)
        in_maps.append({
            "cat_t": cat,
            "offs": _pack_offs(pc, cgg),
        })
    return in_maps


def kernel(inputs_ref, inputs_other, inds_ref, inds_other, weights):
    from concourse.bass_utils import run_bass_kernel_spmd

    inputs_ref = np.asarray(inputs_ref, dtype=np.float32)
    inputs_other = np.asarray(inputs_other, dtype=np.float32)

    per_core, count = _host_prep(inds_ref, inds_other)
    # exact-fit capacity: compile (and cache) the program for the actual
    # worst-core correspondence count, rounded up to whole 128-columns
    max_n = max(len(pc["s"]) for pc in per_core)
    cgg = max(1, -(-max_n // P))
    nc = _get_program(cgg)

    in_maps = _make_in_maps(inputs_ref, inputs_other, per_core, cgg)
    res = run_bass_kernel_spmd(nc, in_maps, core_ids=list(range(NCORES)))
    total = 0.0
    ln19 = float(np.log(np.float32(19.0)))
    for pc, r in zip(per_core, res.results):
        o = np.asarray(r["out"], dtype=np.float64)
        n_pad = cgg * P - len(pc["s"])
        total += o.sum() + n_pad * ln19
    loss = -total / max(count, 1)
    return np.float32(loss)
